# revision 31
# baseline (speedup 1.0000x reference)
"""Causal self-attention (D=1024, H=16, S=2048, B=2) on 8 trn2 cores.

Sharding: core i handles batch b = i // 4 and head-group g = i % 4
(4 heads = 256 model dims per group). Each core computes
    y_partial[b,g] = softmax_causal(Q K^T / 8) V  @ Wo[rows of g]
for its 4 heads; the host sums the 4 group partials per batch and adds bo.

Per-core kernel (bf16 matmul operands, fp32 PSUM accumulation):
  All inputs are repacked HOST-side into partition-contiguous layouts so
  every HBM DMA streams multi-KB descriptors at line rate: xt (x
  transposed) as [128, sb, c, 512], wqkv as [128, 24, 256], wo as
  [128, 2, 1024]. xt is split into 4 per-s-block DMAs on the sync HWDGE
  ring; weights ride the scalar HWDGE ring in parallel; the tiny bias
  tensors go on the gpsimd SWDGE ring so they never block the big rings.
  QKV biases are applied during the psum->SBUF move (DVE
  tensor_scalar_add with a per-partition AP for Q/K, DVE tensor_add with
  a partition_broadcast tile for V): no K=1 seed matmuls.
  Attention per head pair (dc): scoresT = KT^T QT on PE row-group pairs
  (concurrent K=64 matmuls), exp on ACT (table prewarmed at t=0),
  triangular diag masks on Pool, PV accumulation with the ones-column
  softmax-denominator trick (V|1, denom at psum row 64). Normalize:
  ACT-copy denom rows to bf16, PE ones-matmul broadcasts to 64 psum
  partitions, DVE fast-reciprocal, DVE multiplies into AT (e=1 crossing
  partitions via sync-queue DMA).
  Emission interleave: the QKV projection of block sb+1 and the output
  projection of block sb-1 are sliced into 2-matmul chunks and spliced
  between attention T-steps of block sb, so the PE always has fill work
  at fine grain while ACT exp paces the attention. The last block's
  outproj copies alternate ACT/DVE (ACT is idle by then).
"""

import sys

sys.path.insert(0, "/opt/trn_rl_repo")

import ml_dtypes
import numpy as np

import concourse.bass as bass
import concourse.mybir as mybir
import concourse.tile as tile
from concourse import bacc

P = 128
S = 2048
D = 1024
NH = 4                    # heads per core
DH = 64                   # head dim
DPC = NH * DH             # model dims per core = 256
N_CT = D // P             # 8 contraction chunks
N_ST = S // P             # 16 t tiles of 128
N_SB = S // 512           # 4 s blocks of 512
F32 = mybir.dt.float32
BF16 = mybir.dt.bfloat16
SCALE = 1.0 / 8.0         # 1/sqrt(64)

AF = mybir.ActivationFunctionType
ALU = mybir.AluOpType


def build_nc(mm_mode: str = "bf16", stop_after: int = 99,
             skip_norm: bool = False) -> bass.Bass:
    nc = _build(mm_mode, stop_after, skip_norm)
    if not nc.is_finalized():
        nc.finalize()
    return nc


def _build(mm_mode: str, stop_after: int, skip_norm: bool) -> bass.Bass:
    assert mm_mode == "bf16"
    nc = bacc.Bacc("TRN2", target_bir_lowering=False, debug=False,
                   num_devices=8)

    # x transposed + repacked host-side: [128, (sb c s512)]
    xt_d = nc.dram_tensor("xt", [P, N_SB * N_CT * 512], BF16,
                          kind="ExternalInput")
    # w layout host-side: [128, q-dc0 | k-dc0 | q-dc1 | k-dc1 | v] with each
    # qk part (c, 128) and v (c, 256); the first 2048 cols (dc0 Q/K) ride a
    # separate leading DMA so the first projections start ASAP
    wqkv_d = nc.dram_tensor("wqkv", [P, 3 * N_CT * DPC], BF16,
                            kind="ExternalInput")
    wo_d = nc.dram_tensor("wo", [P, 2 * D], BF16, kind="ExternalInput")
    # q/k biases by dc chunk: cols (q-dc0, q-dc1, k-dc0, k-dc1)
    bqk_d = nc.dram_tensor("bqk", [P, 4], F32, kind="ExternalInput")
    bv_d = nc.dram_tensor("bv", [1, DPC], F32, kind="ExternalInput")
    # bf16 partials: host sums 4 of them per batch in fp32
    y_d = nc.dram_tensor("y", [S, D], BF16, kind="ExternalOutput")

    with tile.TileContext(nc) as tc:
        with (
            tc.tile_pool(name="const", bufs=1) as const,
            tc.tile_pool(name="xtp", bufs=1) as xtp,
            tc.tile_pool(name="qkv", bufs=1) as qkv,
            tc.tile_pool(name="atp", bufs=1) as atp,
            tc.tile_pool(name="exw", bufs=4) as exw,
            tc.tile_pool(name="rcpw", bufs=2) as rcpw,
            tc.tile_pool(name="bcw", bufs=2) as bcw,
            tc.tile_pool(name="ysp", bufs=4) as ysp,
            tc.tile_pool(name="psA", bufs=2, space="PSUM") as psA,
            tc.tile_pool(name="psc", bufs=2, space="PSUM") as psc,
            tc.tile_pool(name="ppv", bufs=2, space="PSUM") as ppv,
        ):
            # ---- inputs: weights on the scalar HWDGE ring, x on the sync
            # ring (4 per-s-block DMAs), biases on the gpsimd SWDGE ring.
            # Everything is partition-contiguous: multi-KB descriptors. ----
            bqk_s = const.tile([P, 4], F32)
            nc.gpsimd.dma_start(bqk_s, bqk_d[:, :])
            bv_f = const.tile([1, DPC], F32)
            nc.gpsimd.dma_start(bv_f, bv_d[:, :])
            # wqkv cols: [q-dc0 | k-dc0 | q-dc1 | k-dc1 | v]; q-dc0 and
            # k-dc0 lead so the first projection starts on minimal bytes
            wqkv_s = const.tile([P, 3 * N_CT * DPC], BF16)
            nc.scalar.dma_start(wqkv_s[:, 0:1024], wqkv_d[:, 0:1024])
            xT = xtp.tile([P, N_SB, N_CT, 512], BF16)
            # xt block 0 split in two c-halves: the first Q matmuls start
            # after ~512KB instead of 1MB (4KB descriptors keep line rate)
            for h in range(2):
                nc.sync.dma_start(
                    xT[:, 0, h * 4:(h + 1) * 4, :],
                    xt_d[:, h * 4 * 512:(h + 1) * 4 * 512]
                    .rearrange("p (c s) -> p c s", s=512))
            nc.scalar.dma_start(wqkv_s[:, 1024:2048], wqkv_d[:, 1024:2048])
            nc.sync.dma_start(
                xT[:, 1, :, :],
                xt_d[:, 1 * N_CT * 512:2 * N_CT * 512]
                .rearrange("p (c s) -> p c s", s=512))
            nc.scalar.dma_start(wqkv_s[:, 2048:6144], wqkv_d[:, 2048:6144])
            nc.sync.dma_start(
                xT[:, 2, :, :],
                xt_d[:, 2 * N_CT * 512:3 * N_CT * 512]
                .rearrange("p (c s) -> p c s", s=512))
            # xt block 3 is issued later (fill unit) to keep startup HBM
            # bandwidth for the data the first 25us actually needs
            # wo's DMA is issued later (on the gpsimd queue inside the
            # pipeline) so it never competes with xt/wqkv at startup
            wo_s = const.tile([P, 2, D], BF16)

            def wqk_ap(which, dc, c):  # stationary [128, 128] for Q/K
                off = (dc * 2 + which) * 1024 + c * P
                return wqkv_s[:, off:off + P]

            def wv_ap(c):  # moving [128, 256] for V
                off = 4096 + c * DPC
                return wqkv_s[:, off:off + DPC]

            # bf16 ones rows for the denominator broadcast matmul (sliced
            # at partition 64 to match drow's base partition)
            ones_s = const.tile([P, DH], BF16)
            nc.vector.memset(ones_s, 1.0)
            # V bias broadcast to all partitions (f32) for DVE tensor_add
            bvb = const.tile([P, DPC], F32)
            nc.gpsimd.partition_broadcast(bvb, bv_f)
            # prewarm the ACT exp table during the DMA wait
            warm_i = const.tile([1, 8], F32)
            nc.vector.memset(warm_i, 0.0)
            warm_o = const.tile([1, 8], BF16)
            nc.scalar.activation(warm_o, warm_i, AF.Exp)

            # QT/KT: [128 (head-pair d), dc, s]
            QT = qkv.tile([P, 2, S], BF16)
            KT = qkv.tile([P, 2, S], BF16)
            # V_aug: [t-part, t-chunk, head, 65], col 64 == 1.0 so the PV
            # matmul's psum row 64 accumulates the softmax denominator.
            vaug = qkv.tile([P, N_ST, NH, DH + 1], BF16)
            nc.vector.memset(vaug[:, :, :, DH:DH + 1], 1.0)

            # AT packed by head pairs: [128, dc, s]
            AT = atp.tile([P, 2, S], BF16)

            # ---- fill units: generators yielding one ~2-matmul chunk per
            # next() so attention T-steps can splice them at fine grain ----
            def qk_unit(sb, dc, which):  # which: 0 = Q, 1 = K
                dst = QT if which == 0 else KT
                b_ap = bqk_s[:, which * 2 + dc:which * 2 + dc + 1]
                ps = psA.tile([P, 512], F32, tag="psA")
                for c in range(N_CT):
                    nc.tensor.matmul(
                        ps,
                        wqk_ap(which, dc, c),
                        xT[:, sb, c, :],
                        start=(c == 0),
                        stop=(c == N_CT - 1),
                    )
                    if c % 2 == 1 and c < N_CT - 1:
                        yield
                nc.vector.tensor_scalar_add(
                    dst[:, dc, sb * 512:(sb + 1) * 512], ps, b_ap)
                yield

            def v_unit(tt):
                ps = psA.tile([P, 512], F32, tag="psA")
                pvs = ps[:, 0:DPC]
                sq, sp = divmod(tt, 4)
                for c in range(N_CT):
                    nc.tensor.matmul(
                        pvs,
                        xT[:, sq, c, sp * P:(sp + 1) * P],
                        wv_ap(c),
                        start=(c == 0),
                        stop=(c == N_CT - 1),
                    )
                    if c % 2 == 1 and c < N_CT - 1:
                        yield
                nc.vector.tensor_add(
                    vaug[:, tt, :, 0:DH],
                    pvs.rearrange("p (h u) -> p h u", h=NH),
                    bvb.rearrange("p (h u) -> p h u", h=NH))
                yield

            def outproj_unit(st, last):
                ys = ysp.tile([P, 1024], BF16, tag="ys")
                for eb in range(2):
                    # the tail block draws psums from the (then-idle) ppv
                    # pool so 4 groups can be in flight instead of 2
                    pool = ppv if last else psA
                    ps = pool.tile([P, 512], F32, tag="pv" if last else "psA")
                    for dc in range(2):
                        nc.tensor.matmul(
                            ps,
                            AT[:, dc, st * P:(st + 1) * P],
                            wo_s[:, dc, eb * 512:(eb + 1) * 512],
                            start=(dc == 0),
                            stop=(dc == 1),
                        )
                    # the tail block alternates ACT/DVE (ACT idle by then)
                    if last and eb == 0:
                        nc.scalar.activation(
                            ys[:, 0:512], ps, AF.Copy)
                    else:
                        nc.vector.tensor_copy(
                            ys[:, eb * 512:(eb + 1) * 512], ps)
                    yield
                nc.sync.dma_start(y_d[st * P:(st + 1) * P, :], ys)
                yield

            def chain(gens):
                for g in gens:
                    yield from g

            def qkv_gens(sb, skip_first=0):
                gens = []
                for dc in range(2):
                    for which in range(2):
                        gens.append(qk_unit(sb, dc, which))
                for tt in range(4 * sb, 4 * sb + 4):
                    gens.append(v_unit(tt))
                return gens[skip_first:]

            def outproj_gens(sb, last=False):
                return [outproj_unit(st, last)
                        for st in range(4 * sb, 4 * sb + 4)]

            def _emit_pv(dc, pv_dst, item, t_cnt):
                T, ms, ex = item
                for e in range(2):
                    h = 2 * dc + e
                    nc.tensor.matmul(
                        pv_dst[e][:, ms:512],
                        vaug[:, T, h, :],
                        ex[:, e, ms:512],
                        start=(T == 0),
                        stop=(T == t_cnt - 1),
                    )

            def emit_attention(sb, fill, n_fill):
                # Head pairs (2*dc, 2*dc+1) share each score/exp tile: the
                # two K=64 score matmuls go to PE row-groups 0 and 64
                # (concurrent). fill chunks are spliced between T-steps.
                t_cnt = 4 * sb + 4
                n_steps = 2 * t_cnt
                done = 0
                fi = 0
                si = 0
                for dc in range(2):
                    pvt = [ppv.tile([P, 512], F32, tag="pv",
                                    name=f"pv{sb}_{dc}_{e}")
                           for e in range(2)]
                    # both heads: rows 0..63 = values, row 64 = denominator
                    pv_dst = (pvt[0][0:DH + 1, :], pvt[1][0:DH + 1, :])
                    pend = []  # deferred PV emission: (T, ms, ex)
                    for T in range(t_cnt):
                        k = T - 4 * sb
                        ms = 128 * k if k > 0 else 0
                        sc = psc.tile([P, 2, 512], F32, tag="sc")
                        ex = exw.tile([P, 2, 512], BF16, tag="ex")
                        for e in range(2):  # even/odd head of the pair
                            off = DH * e
                            nc.tensor.matmul(
                                sc[:, e, ms:512],
                                KT[off:off + DH, dc, T * P:(T + 1) * P],
                                QT[off:off + DH, dc,
                                   sb * 512 + ms:(sb + 1) * 512],
                                start=True,
                                stop=True,
                            )
                        nc.scalar.activation(
                            ex[:, :, ms:512], sc[:, :, ms:512],
                            AF.Exp, scale=SCALE,
                        )
                        if k >= 0:  # triangular mask on diagonal chunks
                            nc.gpsimd.affine_select(
                                out=ex[:, :, ms:ms + P],
                                in_=ex[:, :, ms:ms + P],
                                compare_op=ALU.is_ge,
                                fill=0.0,
                                base=0,
                                pattern=[[0, 2], [1, P]],
                                channel_multiplier=-1,
                            )
                        pend.append((T, ms, ex))
                        # defer PV two tiles so its exp is long finished
                        # by the time the PE reaches it (no ramp resets)
                        if len(pend) > 2:
                            _emit_pv(dc, pv_dst, pend.pop(0), t_cnt)
                        si += 1
                        # splice fill chunks evenly across the T-steps
                        while not done and fi + 1 <= si * n_fill / n_steps:
                            try:
                                next(fill)
                                fi += 1
                            except StopIteration:
                                done = 1
                    while pend:
                        _emit_pv(dc, pv_dst, pend.pop(0), t_cnt)

                    if skip_norm:
                        nc.vector.tensor_copy(
                            AT[0:DH, dc, sb * 512:(sb + 1) * 512],
                            pvt[0][0:DH, :])
                        continue
                    # normalize: ACT-copy denom rows to bf16, PE ones-
                    # matmul broadcasts them to 64 psum partitions, DVE
                    # fast-recips, DVE multiplies into AT. Head e=1 goes
                    # first, start to finish, because its result crosses
                    # partitions via a (slow) sync DMA.
                    drow = rcpw.tile([P, 2, 512], BF16, tag="rcp")
                    bcp = psc.tile([P, 2, 512], F32, tag="sc")
                    rcb = bcw.tile([P, 2, 512], F32, tag="bc")
                    for e in (1, 0):
                        nc.scalar.activation(
                            drow[DH:DH + 1, e, :], pvt[e][DH:DH + 1, :],
                            AF.Copy)
                        nc.tensor.matmul(
                            bcp[0:DH, e, :], ones_s[DH:DH + 1, 0:DH],
                            drow[DH:DH + 1, e, :], start=True, stop=True)
                        nc.vector.reciprocal_approx_fast(
                            out=rcb[0:DH, e, :], in_=bcp[0:DH, e, :])
                        if e == 1:
                            att = exw.tile([DH, 512], BF16, tag="att")
                            nc.vector.tensor_mul(
                                att, pvt[1][0:DH, :], rcb[0:DH, 1, :])
                            nc.sync.dma_start(
                                AT[DH:P, dc, sb * 512:(sb + 1) * 512], att)
                        else:
                            nc.vector.tensor_mul(
                                AT[0:DH, dc, sb * 512:(sb + 1) * 512],
                                pvt[0][0:DH, :], rcb[0:DH, 0, :])
                # drain any leftover fill work
                while not done:
                    try:
                        next(fill)
                    except StopIteration:
                        done = 1

            def wo_dma_unit():
                nc.gpsimd.dma_start(
                    wo_s, wo_d.rearrange("p (dc e) -> p dc e", e=D))
                yield

            def xt3_dma_unit():
                nc.sync.dma_start(
                    xT[:, 3, :, :],
                    xt_d[:, 3 * N_CT * 512:4 * N_CT * 512]
                    .rearrange("p (c s) -> p c s", s=512))
                yield

            # ---- pipeline ----
            # minimal prefix: Q/K of dc0 only, so the first score matmul
            # (and hence the first exp) lands as early as possible. All
            # remaining projections ride the fill stream: block sb+1's
            # projections during attention(sb); ALL out-projections during
            # attention(3), where exp dominates and the PE would starve.
            prefix = [qk_unit(0, 0, 0), qk_unit(0, 0, 1)]
            for g in prefix:
                for _ in g:
                    pass
            # fill chunk counts: qk_unit = 4, v_unit = 2, outproj_unit = 3
            for sb in range(N_SB):
                gens = []
                n_fill = 0
                if sb == 0:
                    gens += [wo_dma_unit(),
                             v_unit(0), v_unit(1), v_unit(2), v_unit(3),
                             qk_unit(0, 1, 0), qk_unit(0, 1, 1)]
                    n_fill += 1 + 4 * 4 + 2 * 4
                    gens += qkv_gens(1)
                    n_fill += 32
                elif sb == 1:
                    gens += [xt3_dma_unit()]
                    gens += qkv_gens(2)
                    n_fill += 1 + 32
                elif sb == 2:
                    gens += qkv_gens(3)
                    n_fill += 32
                else:
                    for b in range(3):
                        gens += outproj_gens(b)
                    n_fill += 36
                if stop_after <= 1 and sb == 0:
                    for _ in chain(gens):
                        pass
                    break
                emit_attention(sb, chain(gens), n_fill)
            if stop_after > 1:
                for _ in chain(outproj_gens(N_SB - 1, last=True)):
                    pass

    return nc


_NC_CACHE = {}


def _get_nc(mm_mode="bf16"):
    if mm_mode not in _NC_CACHE:
        _NC_CACHE[mm_mode] = build_nc(mm_mode=mm_mode)
    return _NC_CACHE[mm_mode]


MM_MODE = "bf16"


def make_in_maps(x, Wq, bq, Wk, bk, Wv, bv, Wo, mm_mode=None):
    """Per-core input dicts: core i -> (batch i//4, head-group i%4)."""
    bf = ml_dtypes.bfloat16
    in_maps = []
    for core in range(8):
        b, g = core // 4, core % 4
        sl = slice(g * DPC, (g + 1) * DPC)
        # repack to the SBUF layouts (partition-contiguous rows)
        # wqkv cols: [q-dc0 | k-dc0 | q-dc1 | k-dc1 | v], qk parts (c, 128)
        def qk_part(W, dc):
            return W[:, sl].reshape(N_CT, P, DPC)[:, :, dc * P:(dc + 1) * P] \
                    .transpose(1, 0, 2).reshape(P, N_CT * P)
        v_part = Wv[:, sl].reshape(N_CT, P, DPC).transpose(1, 0, 2) \
                          .reshape(P, N_CT * DPC)
        wqkv_p = np.concatenate(
            [qk_part(Wq, 0), qk_part(Wk, 0),
             qk_part(Wq, 1), qk_part(Wk, 1), v_part], axis=1)
        xt = x[b].T  # [D, S]
        xt_p = xt.reshape(N_CT, P, N_SB, 512).transpose(1, 2, 0, 3) \
                 .reshape(P, N_SB * N_CT * 512)
        wo_p = Wo[sl, :].reshape(2, P, D).transpose(1, 0, 2).reshape(P, 2 * D)
        bqk = np.stack([bq[sl][0:P], bq[sl][P:2 * P],
                        bk[sl][0:P], bk[sl][P:2 * P]], axis=1)  # [128, 4]
        in_maps.append({
            "xt": np.ascontiguousarray(xt_p).astype(bf),
            "wqkv": np.ascontiguousarray(wqkv_p).astype(bf),
            "wo": np.ascontiguousarray(wo_p).astype(bf),
            "bqk": np.ascontiguousarray(bqk).astype(np.float32),
            "bv": np.ascontiguousarray(bv[sl][None, :]).astype(np.float32),
        })
    return in_maps


def combine_results(results, bo):
    out = np.zeros((2, S, D), dtype=np.float32)
    for core in range(8):
        out[core // 4] += results[core]["y"]
    out += bo.astype(np.float32)
    return out


_RUNNER_CACHE = {}


def get_runner(mm_mode=None):
    """Build (once) a jitted 8-core runner; returns fn(in_maps) -> results."""
    mode = mm_mode or MM_MODE
    if mode in _RUNNER_CACHE:
        return _RUNNER_CACHE[mode]

    import jax
    from jax.sharding import Mesh, PartitionSpec
    from jax.experimental.shard_map import shard_map
    from concourse import bass2jax, mybir as _mb

    nc = _get_nc(mode)
    bass2jax.install_neuronx_cc_hook()

    pname = nc.partition_id_tensor.name if nc.partition_id_tensor else None
    in_names, out_names, out_avals = [], [], []
    for alloc in nc.m.functions[0].allocations:
        if not isinstance(alloc, _mb.MemoryLocationSet):
            continue
        name = alloc.memorylocations[0].name
        if alloc.kind == "ExternalInput":
            if name != pname:
                in_names.append(name)
        elif alloc.kind == "ExternalOutput":
            out_names.append(name)
            out_avals.append(jax.core.ShapedArray(
                tuple(alloc.tensor_shape), _mb.dt.np(alloc.dtype)))
    n_params = len(in_names)
    all_names = in_names + out_names
    if pname is not None:
        all_names = all_names + [pname]

    def _body(*args):
        operands = list(args)
        if pname is not None:
            operands.append(bass2jax.partition_id_tensor())
        outs = bass2jax._bass_exec_p.bind(
            *operands,
            out_avals=tuple(out_avals),
            in_names=tuple(all_names),
            out_names=tuple(out_names),
            lowering_input_output_aliases=(),
            sim_require_finite=True,
            sim_require_nnan=True,
            nc=nc,
        )
        return tuple(outs)

    devices = jax.devices()[:8]
    mesh = Mesh(np.asarray(devices), ("core",))
    sharded = jax.jit(
        shard_map(_body, mesh=mesh,
                  in_specs=(PartitionSpec("core"),) * (n_params + len(out_names)),
                  out_specs=(PartitionSpec("core"),) * len(out_names),
                  check_rep=False),
        keep_unused=True,
    )

    from jax.sharding import NamedSharding
    zero_outs = [
        jax.device_put(
            np.zeros((8 * a.shape[0], *a.shape[1:]), a.dtype),
            NamedSharding(mesh, PartitionSpec("core")),
        )
        for a in out_avals
    ]

    def run(in_maps):
        concat_in = [
            np.concatenate([np.asarray(m[name]) for m in in_maps], axis=0)
            for name in in_names
        ]
        out_arrs = sharded(*concat_in, *zero_outs)
        return [
            {name: np.asarray(out_arrs[i]).reshape(8, *out_avals[i].shape)[c]
             for i, name in enumerate(out_names)}
            for c in range(8)
        ]

    run.sharded = sharded
    run.in_names = in_names
    run.out_names = out_names
    run.out_avals = out_avals
    run.zero_outs = zero_outs
    _RUNNER_CACHE[mode] = run
    return run


def kernel(x, Wq, bq, Wk, bk, Wv, bv, Wo, bo, **_ignored):
    x = np.asarray(x, dtype=np.float32)
    in_maps = make_in_maps(
        x,
        np.asarray(Wq, np.float32), np.asarray(bq, np.float32),
        np.asarray(Wk, np.float32), np.asarray(bk, np.float32),
        np.asarray(Wv, np.float32), np.asarray(bv, np.float32),
        np.asarray(Wo, np.float32),
    )
    try:
        results = get_runner(MM_MODE)(in_maps)
    except Exception:
        # fallback: stock SPMD runner (slower dispatch, same NEFF)
        from concourse.bass_utils import run_bass_kernel_spmd
        results = run_bass_kernel_spmd(
            _get_nc(MM_MODE), in_maps, core_ids=list(range(8))).results
    return combine_results(results, np.asarray(bo, np.float32))


# revision 33
# speedup vs baseline: 1.0215x; 1.0215x over previous
"""Causal self-attention (D=1024, H=16, S=2048, B=2) on 8 trn2 cores.

Sharding: core i handles batch b = i // 4 and head-group g = i % 4
(4 heads = 256 model dims per group). Each core computes
    y_partial[b,g] = softmax_causal(Q K^T / 8) V  @ Wo[rows of g]
for its 4 heads; the host sums the 4 group partials per batch and adds bo.

Per-core kernel (bf16 matmul operands, fp32 PSUM accumulation):
  All inputs are repacked HOST-side into partition-contiguous layouts so
  every HBM DMA streams multi-KB descriptors at line rate: xt (x
  transposed) as [128, sb, c, 512], wqkv as [128, 24, 256], wo as
  [128, 2, 1024]. xt is split into 4 per-s-block DMAs on the sync HWDGE
  ring; weights ride the scalar HWDGE ring in parallel; the tiny bias
  tensors go on the gpsimd SWDGE ring so they never block the big rings.
  QKV biases are applied during the psum->SBUF move (DVE
  tensor_scalar_add with a per-partition AP for Q/K, DVE tensor_add with
  a partition_broadcast tile for V): no K=1 seed matmuls.
  Attention per head pair (dc): scoresT = KT^T QT on PE row-group pairs
  (concurrent K=64 matmuls), exp on ACT (table prewarmed at t=0),
  triangular diag masks on Pool, PV accumulation with the ones-column
  softmax-denominator trick (V|1, denom at psum row 64). Normalize:
  ACT-copy denom rows to bf16, PE ones-matmul broadcasts to 64 psum
  partitions, DVE fast-reciprocal, DVE multiplies into AT (e=1 crossing
  partitions via sync-queue DMA).
  Emission interleave: the QKV projection of block sb+1 and the output
  projection of block sb-1 are sliced into 2-matmul chunks and spliced
  between attention T-steps of block sb, so the PE always has fill work
  at fine grain while ACT exp paces the attention. The last block's
  outproj copies alternate ACT/DVE (ACT is idle by then).
"""

import sys

sys.path.insert(0, "/opt/trn_rl_repo")

import ml_dtypes
import numpy as np

import concourse.bass as bass
import concourse.mybir as mybir
import concourse.tile as tile
from concourse import bacc

P = 128
S = 2048
D = 1024
NH = 4                    # heads per core
DH = 64                   # head dim
DPC = NH * DH             # model dims per core = 256
N_CT = D // P             # 8 contraction chunks
N_ST = S // P             # 16 t tiles of 128
N_SB = S // 512           # 4 s blocks of 512
F32 = mybir.dt.float32
BF16 = mybir.dt.bfloat16
SCALE = 1.0 / 8.0         # 1/sqrt(64)

AF = mybir.ActivationFunctionType
ALU = mybir.AluOpType


def build_nc(mm_mode: str = "bf16", stop_after: int = 99,
             skip_norm: bool = False) -> bass.Bass:
    nc = _build(mm_mode, stop_after, skip_norm)
    if not nc.is_finalized():
        nc.finalize()
    return nc


def _build(mm_mode: str, stop_after: int, skip_norm: bool) -> bass.Bass:
    assert mm_mode == "bf16"
    nc = bacc.Bacc("TRN2", target_bir_lowering=False, debug=False,
                   num_devices=8)

    # x transposed + repacked host-side: [128, (sb c s512)]
    xt_d = nc.dram_tensor("xt", [P, N_SB * N_CT * 512], BF16,
                          kind="ExternalInput")
    # w layout host-side: [128, q-dc0 | k-dc0 | q-dc1 | k-dc1 | v] with each
    # qk part (c, 128) and v (c, 256); the first 2048 cols (dc0 Q/K) ride a
    # separate leading DMA so the first projections start ASAP
    wqkv_d = nc.dram_tensor("wqkv", [P, 3 * N_CT * DPC], BF16,
                            kind="ExternalInput")
    wo_d = nc.dram_tensor("wo", [P, 2 * D], BF16, kind="ExternalInput")
    # q/k biases by dc chunk: cols (q-dc0, q-dc1, k-dc0, k-dc1)
    bqk_d = nc.dram_tensor("bqk", [P, 4], F32, kind="ExternalInput")
    bv_d = nc.dram_tensor("bv", [1, DPC], F32, kind="ExternalInput")
    # bf16 partials: host sums 4 of them per batch in fp32
    y_d = nc.dram_tensor("y", [S, D], BF16, kind="ExternalOutput")

    with tile.TileContext(nc) as tc:
        with (
            tc.tile_pool(name="const", bufs=1) as const,
            tc.tile_pool(name="xtp", bufs=1) as xtp,
            tc.tile_pool(name="qkv", bufs=1) as qkv,
            tc.tile_pool(name="atp", bufs=1) as atp,
            tc.tile_pool(name="exw", bufs=4) as exw,
            tc.tile_pool(name="rcpw", bufs=2) as rcpw,
            tc.tile_pool(name="bcw", bufs=2) as bcw,
            tc.tile_pool(name="ysp", bufs=4) as ysp,
            tc.tile_pool(name="psA", bufs=2, space="PSUM") as psA,
            tc.tile_pool(name="psc", bufs=2, space="PSUM") as psc,
            tc.tile_pool(name="ppv", bufs=2, space="PSUM") as ppv,
        ):
            # ---- inputs: weights on the scalar HWDGE ring, x on the sync
            # ring (4 per-s-block DMAs), biases on the gpsimd SWDGE ring.
            # Everything is partition-contiguous: multi-KB descriptors. ----
            bqk_s = const.tile([P, 4], F32)
            nc.gpsimd.dma_start(bqk_s, bqk_d[:, :])
            bv_f = const.tile([1, DPC], F32)
            nc.gpsimd.dma_start(bv_f, bv_d[:, :])
            # wqkv cols: [q-dc0 | k-dc0 | q-dc1 | k-dc1 | v]; q-dc0 and
            # k-dc0 lead so the first projection starts on minimal bytes
            wqkv_s = const.tile([P, 3 * N_CT * DPC], BF16)
            nc.scalar.dma_start(wqkv_s[:, 0:1024], wqkv_d[:, 0:1024])
            xT = xtp.tile([P, N_SB, N_CT, 512], BF16)
            # xt block 0 split in two c-halves: the first Q matmuls start
            # after ~512KB instead of 1MB (4KB descriptors keep line rate)
            for h in range(2):
                nc.sync.dma_start(
                    xT[:, 0, h * 4:(h + 1) * 4, :],
                    xt_d[:, h * 4 * 512:(h + 1) * 4 * 512]
                    .rearrange("p (c s) -> p c s", s=512))
            nc.scalar.dma_start(wqkv_s[:, 1024:2048], wqkv_d[:, 1024:2048])
            nc.sync.dma_start(
                xT[:, 1, :, :],
                xt_d[:, 1 * N_CT * 512:2 * N_CT * 512]
                .rearrange("p (c s) -> p c s", s=512))
            nc.scalar.dma_start(wqkv_s[:, 2048:6144], wqkv_d[:, 2048:6144])
            nc.sync.dma_start(
                xT[:, 2, :, :],
                xt_d[:, 2 * N_CT * 512:3 * N_CT * 512]
                .rearrange("p (c s) -> p c s", s=512))
            # xt block 3 is issued later (fill unit) to keep startup HBM
            # bandwidth for the data the first 25us actually needs
            # wo's DMA is issued later (on the gpsimd queue inside the
            # pipeline) so it never competes with xt/wqkv at startup
            wo_s = const.tile([P, 2, D], BF16)

            def wqk_ap(which, dc, c):  # stationary [128, 128] for Q/K
                off = (dc * 2 + which) * 1024 + c * P
                return wqkv_s[:, off:off + P]

            def wv_ap(c):  # moving [128, 256] for V
                off = 4096 + c * DPC
                return wqkv_s[:, off:off + DPC]

            # bf16 ones rows for the denominator broadcast matmul (sliced
            # at partition 64 to match drow's base partition)
            ones_s = const.tile([P, DH], BF16)
            nc.vector.memset(ones_s, 1.0)
            # V bias broadcast to all partitions (f32) for DVE tensor_add
            bvb = const.tile([P, DPC], F32)
            nc.gpsimd.partition_broadcast(bvb, bv_f)
            # prewarm the ACT exp table during the DMA wait
            warm_i = const.tile([1, 8], F32)
            nc.vector.memset(warm_i, 0.0)
            warm_o = const.tile([1, 8], BF16)
            nc.scalar.activation(warm_o, warm_i, AF.Exp)

            # QT/KT: [128 (head-pair d), dc, s]
            QT = qkv.tile([P, 2, S], BF16)
            KT = qkv.tile([P, 2, S], BF16)
            # V_aug: [t-part, t-chunk, head, 65], col 64 == 1.0 so the PV
            # matmul's psum row 64 accumulates the softmax denominator.
            vaug = qkv.tile([P, N_ST, NH, DH + 1], BF16)
            nc.vector.memset(vaug[:, :, :, DH:DH + 1], 1.0)

            # AT packed by head pairs: [128, dc, s]
            AT = atp.tile([P, 2, S], BF16)

            # ---- fill units: generators yielding one ~2-matmul chunk per
            # next() so attention T-steps can splice them at fine grain ----
            def qk_unit(sb, dc, which):  # which: 0 = Q, 1 = K
                dst = QT if which == 0 else KT
                b_ap = bqk_s[:, which * 2 + dc:which * 2 + dc + 1]
                ps = psA.tile([P, 512], F32, tag="psA")
                for c in range(N_CT):
                    nc.tensor.matmul(
                        ps,
                        wqk_ap(which, dc, c),
                        xT[:, sb, c, :],
                        start=(c == 0),
                        stop=(c == N_CT - 1),
                    )
                    if c % 2 == 1 and c < N_CT - 1:
                        yield
                nc.vector.tensor_scalar_add(
                    dst[:, dc, sb * 512:(sb + 1) * 512], ps, b_ap)
                yield

            def v_unit(tt):
                ps = psA.tile([P, 512], F32, tag="psA")
                pvs = ps[:, 0:DPC]
                sq, sp = divmod(tt, 4)
                for c in range(N_CT):
                    nc.tensor.matmul(
                        pvs,
                        xT[:, sq, c, sp * P:(sp + 1) * P],
                        wv_ap(c),
                        start=(c == 0),
                        stop=(c == N_CT - 1),
                    )
                    if c % 4 == 3 and c < N_CT - 1:
                        yield
                nc.vector.tensor_add(
                    vaug[:, tt, :, 0:DH],
                    pvs.rearrange("p (h u) -> p h u", h=NH),
                    bvb.rearrange("p (h u) -> p h u", h=NH))
                yield

            def outproj_unit(st, last):
                ys = ysp.tile([P, 1024], BF16, tag="ys")
                for eb in range(2):
                    # the tail block draws psums from the (then-idle) ppv
                    # pool so 4 groups can be in flight instead of 2
                    pool = ppv if last else psA
                    ps = pool.tile([P, 512], F32, tag="pv" if last else "psA")
                    for dc in range(2):
                        nc.tensor.matmul(
                            ps,
                            AT[:, dc, st * P:(st + 1) * P],
                            wo_s[:, dc, eb * 512:(eb + 1) * 512],
                            start=(dc == 0),
                            stop=(dc == 1),
                        )
                    # the tail block alternates ACT/DVE (ACT idle by then)
                    if last and eb == 0:
                        nc.scalar.activation(
                            ys[:, 0:512], ps, AF.Copy)
                    else:
                        nc.vector.tensor_copy(
                            ys[:, eb * 512:(eb + 1) * 512], ps)
                    yield
                nc.sync.dma_start(y_d[st * P:(st + 1) * P, :], ys)
                yield

            def chain(gens):
                for g in gens:
                    yield from g

            def qkv_gens(sb, skip_first=0):
                gens = []
                for dc in range(2):
                    for which in range(2):
                        gens.append(qk_unit(sb, dc, which))
                for tt in range(4 * sb, 4 * sb + 4):
                    gens.append(v_unit(tt))
                return gens[skip_first:]

            def outproj_gens(sb, last=False):
                return [outproj_unit(st, last)
                        for st in range(4 * sb, 4 * sb + 4)]

            def _emit_pv(dc, pv_dst, item, t_cnt):
                T, ms, ex = item
                for e in range(2):
                    h = 2 * dc + e
                    nc.tensor.matmul(
                        pv_dst[e][:, ms:512],
                        vaug[:, T, h, :],
                        ex[:, e, ms:512],
                        start=(T == 0),
                        stop=(T == t_cnt - 1),
                    )

            def emit_attention(sb, fill, n_fill):
                # Head pairs (2*dc, 2*dc+1) share each score/exp tile: the
                # two K=64 score matmuls go to PE row-groups 0 and 64
                # (concurrent). fill chunks are spliced between T-steps.
                t_cnt = 4 * sb + 4
                n_steps = 2 * t_cnt
                done = 0
                fi = 0
                si = 0
                for dc in range(2):
                    pvt = [ppv.tile([P, 512], F32, tag="pv",
                                    name=f"pv{sb}_{dc}_{e}")
                           for e in range(2)]
                    # both heads: rows 0..63 = values, row 64 = denominator
                    pv_dst = (pvt[0][0:DH + 1, :], pvt[1][0:DH + 1, :])
                    pend = []  # deferred PV emission: (T, ms, ex)
                    for T in range(t_cnt):
                        k = T - 4 * sb
                        ms = 128 * k if k > 0 else 0
                        sc = psc.tile([P, 2, 512], F32, tag="sc")
                        ex = exw.tile([P, 2, 512], BF16, tag="ex")
                        for e in range(2):  # even/odd head of the pair
                            off = DH * e
                            nc.tensor.matmul(
                                sc[:, e, ms:512],
                                KT[off:off + DH, dc, T * P:(T + 1) * P],
                                QT[off:off + DH, dc,
                                   sb * 512 + ms:(sb + 1) * 512],
                                start=True,
                                stop=True,
                            )
                        nc.scalar.activation(
                            ex[:, :, ms:512], sc[:, :, ms:512],
                            AF.Exp, scale=SCALE,
                        )
                        if k >= 0:  # triangular mask on diagonal chunks
                            nc.gpsimd.affine_select(
                                out=ex[:, :, ms:ms + P],
                                in_=ex[:, :, ms:ms + P],
                                compare_op=ALU.is_ge,
                                fill=0.0,
                                base=0,
                                pattern=[[0, 2], [1, P]],
                                channel_multiplier=-1,
                            )
                        pend.append((T, ms, ex))
                        # defer PV two tiles so its exp is long finished
                        # by the time the PE reaches it (no ramp resets)
                        if len(pend) > 2:
                            _emit_pv(dc, pv_dst, pend.pop(0), t_cnt)
                        si += 1
                        # splice fill chunks evenly across the T-steps
                        while not done and fi + 1 <= si * n_fill / n_steps:
                            try:
                                next(fill)
                                fi += 1
                            except StopIteration:
                                done = 1
                    while pend:
                        _emit_pv(dc, pv_dst, pend.pop(0), t_cnt)

                    if skip_norm:
                        nc.vector.tensor_copy(
                            AT[0:DH, dc, sb * 512:(sb + 1) * 512],
                            pvt[0][0:DH, :])
                        continue
                    # normalize: ACT-copy denom rows to bf16, PE ones-
                    # matmul broadcasts them to 64 psum partitions, DVE
                    # fast-recips, DVE multiplies into AT. Head e=1 goes
                    # first, start to finish, because its result crosses
                    # partitions via a (slow) sync DMA.
                    drow = rcpw.tile([P, 2, 512], BF16, tag="rcp")
                    bcp = psc.tile([P, 2, 512], F32, tag="sc")
                    rcb = bcw.tile([P, 2, 512], F32, tag="bc")
                    for e in (1, 0):
                        nc.scalar.activation(
                            drow[DH:DH + 1, e, :], pvt[e][DH:DH + 1, :],
                            AF.Copy)
                        nc.tensor.matmul(
                            bcp[0:DH, e, :], ones_s[DH:DH + 1, 0:DH],
                            drow[DH:DH + 1, e, :], start=True, stop=True)
                        nc.vector.reciprocal_approx_fast(
                            out=rcb[0:DH, e, :], in_=bcp[0:DH, e, :])
                        if e == 1:
                            att = exw.tile([DH, 512], BF16, tag="att")
                            nc.vector.tensor_mul(
                                att, pvt[1][0:DH, :], rcb[0:DH, 1, :])
                            nc.sync.dma_start(
                                AT[DH:P, dc, sb * 512:(sb + 1) * 512], att)
                        else:
                            nc.vector.tensor_mul(
                                AT[0:DH, dc, sb * 512:(sb + 1) * 512],
                                pvt[0][0:DH, :], rcb[0:DH, 0, :])
                # drain any leftover fill work
                while not done:
                    try:
                        next(fill)
                    except StopIteration:
                        done = 1

            def wo_dma_unit():
                nc.gpsimd.dma_start(
                    wo_s, wo_d.rearrange("p (dc e) -> p dc e", e=D))
                yield

            def xt3_dma_unit():
                nc.sync.dma_start(
                    xT[:, 3, :, :],
                    xt_d[:, 3 * N_CT * 512:4 * N_CT * 512]
                    .rearrange("p (c s) -> p c s", s=512))
                yield

            # ---- pipeline ----
            # minimal prefix: Q/K of dc0 only, so the first score matmul
            # (and hence the first exp) lands as early as possible. All
            # remaining projections ride the fill stream: block sb+1's
            # projections during attention(sb); ALL out-projections during
            # attention(3), where exp dominates and the PE would starve.
            prefix = [qk_unit(0, 0, 0), qk_unit(0, 0, 1)]
            for g in prefix:
                for _ in g:
                    pass
            # fill chunk counts: qk_unit = 4, v_unit = 2, outproj_unit = 3
            for sb in range(N_SB):
                gens = []
                n_fill = 0
                if sb == 0:
                    gens += [wo_dma_unit(),
                             v_unit(0), v_unit(1), v_unit(2), v_unit(3),
                             qk_unit(0, 1, 0), qk_unit(0, 1, 1)]
                    n_fill += 1 + 4 * 2 + 2 * 4
                    gens += qkv_gens(1)
                    n_fill += 24
                elif sb == 1:
                    gens += [xt3_dma_unit()]
                    gens += qkv_gens(2)
                    n_fill += 1 + 24
                elif sb == 2:
                    gens += qkv_gens(3)
                    n_fill += 24
                else:
                    for b in range(3):
                        gens += outproj_gens(b)
                    n_fill += 36
                if stop_after <= 1 and sb == 0:
                    for _ in chain(gens):
                        pass
                    break
                emit_attention(sb, chain(gens), n_fill)
            if stop_after > 1:
                for _ in chain(outproj_gens(N_SB - 1, last=True)):
                    pass

    return nc


_NC_CACHE = {}


def _get_nc(mm_mode="bf16"):
    if mm_mode not in _NC_CACHE:
        _NC_CACHE[mm_mode] = build_nc(mm_mode=mm_mode)
    return _NC_CACHE[mm_mode]


MM_MODE = "bf16"


def make_in_maps(x, Wq, bq, Wk, bk, Wv, bv, Wo, mm_mode=None):
    """Per-core input dicts: core i -> (batch i//4, head-group i%4)."""
    bf = ml_dtypes.bfloat16
    in_maps = []
    for core in range(8):
        b, g = core // 4, core % 4
        sl = slice(g * DPC, (g + 1) * DPC)
        # repack to the SBUF layouts (partition-contiguous rows)
        # wqkv cols: [q-dc0 | k-dc0 | q-dc1 | k-dc1 | v], qk parts (c, 128)
        def qk_part(W, dc):
            return W[:, sl].reshape(N_CT, P, DPC)[:, :, dc * P:(dc + 1) * P] \
                    .transpose(1, 0, 2).reshape(P, N_CT * P)
        v_part = Wv[:, sl].reshape(N_CT, P, DPC).transpose(1, 0, 2) \
                          .reshape(P, N_CT * DPC)
        wqkv_p = np.concatenate(
            [qk_part(Wq, 0), qk_part(Wk, 0),
             qk_part(Wq, 1), qk_part(Wk, 1), v_part], axis=1)
        xt = x[b].T  # [D, S]
        xt_p = xt.reshape(N_CT, P, N_SB, 512).transpose(1, 2, 0, 3) \
                 .reshape(P, N_SB * N_CT * 512)
        wo_p = Wo[sl, :].reshape(2, P, D).transpose(1, 0, 2).reshape(P, 2 * D)
        bqk = np.stack([bq[sl][0:P], bq[sl][P:2 * P],
                        bk[sl][0:P], bk[sl][P:2 * P]], axis=1)  # [128, 4]
        in_maps.append({
            "xt": np.ascontiguousarray(xt_p).astype(bf),
            "wqkv": np.ascontiguousarray(wqkv_p).astype(bf),
            "wo": np.ascontiguousarray(wo_p).astype(bf),
            "bqk": np.ascontiguousarray(bqk).astype(np.float32),
            "bv": np.ascontiguousarray(bv[sl][None, :]).astype(np.float32),
        })
    return in_maps


def combine_results(results, bo):
    out = np.zeros((2, S, D), dtype=np.float32)
    for core in range(8):
        out[core // 4] += results[core]["y"]
    out += bo.astype(np.float32)
    return out


_RUNNER_CACHE = {}


def get_runner(mm_mode=None):
    """Build (once) a jitted 8-core runner; returns fn(in_maps) -> results."""
    mode = mm_mode or MM_MODE
    if mode in _RUNNER_CACHE:
        return _RUNNER_CACHE[mode]

    import jax
    from jax.sharding import Mesh, PartitionSpec
    from jax.experimental.shard_map import shard_map
    from concourse import bass2jax, mybir as _mb

    nc = _get_nc(mode)
    bass2jax.install_neuronx_cc_hook()

    pname = nc.partition_id_tensor.name if nc.partition_id_tensor else None
    in_names, out_names, out_avals = [], [], []
    for alloc in nc.m.functions[0].allocations:
        if not isinstance(alloc, _mb.MemoryLocationSet):
            continue
        name = alloc.memorylocations[0].name
        if alloc.kind == "ExternalInput":
            if name != pname:
                in_names.append(name)
        elif alloc.kind == "ExternalOutput":
            out_names.append(name)
            out_avals.append(jax.core.ShapedArray(
                tuple(alloc.tensor_shape), _mb.dt.np(alloc.dtype)))
    n_params = len(in_names)
    all_names = in_names + out_names
    if pname is not None:
        all_names = all_names + [pname]

    def _body(*args):
        operands = list(args)
        if pname is not None:
            operands.append(bass2jax.partition_id_tensor())
        outs = bass2jax._bass_exec_p.bind(
            *operands,
            out_avals=tuple(out_avals),
            in_names=tuple(all_names),
            out_names=tuple(out_names),
            lowering_input_output_aliases=(),
            sim_require_finite=True,
            sim_require_nnan=True,
            nc=nc,
        )
        return tuple(outs)

    devices = jax.devices()[:8]
    mesh = Mesh(np.asarray(devices), ("core",))
    sharded = jax.jit(
        shard_map(_body, mesh=mesh,
                  in_specs=(PartitionSpec("core"),) * (n_params + len(out_names)),
                  out_specs=(PartitionSpec("core"),) * len(out_names),
                  check_rep=False),
        keep_unused=True,
    )

    from jax.sharding import NamedSharding
    zero_outs = [
        jax.device_put(
            np.zeros((8 * a.shape[0], *a.shape[1:]), a.dtype),
            NamedSharding(mesh, PartitionSpec("core")),
        )
        for a in out_avals
    ]

    def run(in_maps):
        concat_in = [
            np.concatenate([np.asarray(m[name]) for m in in_maps], axis=0)
            for name in in_names
        ]
        out_arrs = sharded(*concat_in, *zero_outs)
        return [
            {name: np.asarray(out_arrs[i]).reshape(8, *out_avals[i].shape)[c]
             for i, name in enumerate(out_names)}
            for c in range(8)
        ]

    run.sharded = sharded
    run.in_names = in_names
    run.out_names = out_names
    run.out_avals = out_avals
    run.zero_outs = zero_outs
    _RUNNER_CACHE[mode] = run
    return run


def kernel(x, Wq, bq, Wk, bk, Wv, bv, Wo, bo, **_ignored):
    x = np.asarray(x, dtype=np.float32)
    in_maps = make_in_maps(
        x,
        np.asarray(Wq, np.float32), np.asarray(bq, np.float32),
        np.asarray(Wk, np.float32), np.asarray(bk, np.float32),
        np.asarray(Wv, np.float32), np.asarray(bv, np.float32),
        np.asarray(Wo, np.float32),
    )
    try:
        results = get_runner(MM_MODE)(in_maps)
    except Exception:
        # fallback: stock SPMD runner (slower dispatch, same NEFF)
        from concourse.bass_utils import run_bass_kernel_spmd
        results = run_bass_kernel_spmd(
            _get_nc(MM_MODE), in_maps, core_ids=list(range(8))).results
    return combine_results(results, np.asarray(bo, np.float32))
